# revision 28
# baseline (speedup 1.0000x reference)
"""MHA (projections + masked softmax attention) on 8 NeuronCores.

Data-parallel over batch (B=8 -> 1 batch element per core, no collectives).
bf16 matmul operands (fp32 PSUM accumulation + fp32 softmax normalization).

Per core, transposed layout:
  QT = Wq^T @ x_q^T   [D, Sq]   (lhsT = Wq natural, rhs = x_q^T from host)
  KT = Wk^T @ x_k^T   [D, Sk]
  V  = x_v  @ Wv      [Sk, D]   (lhsT = x_v^T chunk, rhs = Wv natural)

Attention processed in head PAIRS (2h, 2h+1) sharing one 128-partition
qt/kt chunk: the two heads' scores matmuls have K=64 contraction at PE
row offsets 0 and 64 (tile_position row-tiling) and execute concurrently
on the PE array. Scores in "transposed" layout S^T[k, q]:
  S^T = KT_h_chunk.T @ QT_h                (k on partitions, q free)
  masking: e = exp(0.125*s) first, then e *= valid01 (bf16 0/1 mask) on
  the ragged [c0:cv) column range only -- off the PSUM-recycle chain.
  O^T[d,q] & Z[q] in ONE accumulating matmul: lhsT = [V_h | ones] (65 cols)
  final: O = transpose(O^T) * (1/Z) per 128-query block; ONE strided
  bf16 DMA per head (issued from the Pool engine DGE to keep SP free).

Host: transposes, sort queries by valid_len (column-suffix skipping of
fully-masked key chunks + narrow predication ranges), uint8 mask, exact
fixup of valid_len==0 rows (reference -> uniform softmax -> mean(value)@Wv).
"""

import os
import sys

if "/opt/trn_rl_repo" not in sys.path:
    sys.path.insert(0, "/opt/trn_rl_repo")

import numpy as np

ABLATE = set(os.environ.get("ABLATE", "").split(","))

B, S, D, H = 8, 1024, 1024, 16
DH = D // H  # 64
P = 128
KC = S // P  # 8 key chunks
DC = D // P  # 8 hidden chunks
N_CORES = 8
NEG = -480.0  # exp(0.125 * -480) = exp(-60) ~= 8.8e-27
TRS = 128  # per-chunk column stride in the transpose staging tile


def _build_nc(col_start, pred_end, reps=1):
    """col_start[kc]: first sorted-q column (mult of 128, 0..1024) needing
    key-chunk kc (1024 = chunk skipped). pred_end[kc]: end (exclusive, mult
    of 32) of the mask-predication range. Unions over cores. col_start[0]
    must be 0."""
    from contextlib import ExitStack

    import concourse.mybir as mybir
    import concourse.tile as tile
    from concourse import bacc
    from concourse.masks import make_identity

    fp32 = mybir.dt.float32
    bf16 = mybir.dt.bfloat16
    u8 = mybir.dt.uint8
    AF = mybir.ActivationFunctionType

    nc = bacc.Bacc(
        "TRN2",
        target_bir_lowering=False,
        debug=False,
        enable_asserts=False,
        num_devices=N_CORES,
    )

    xqT = nc.dram_tensor("xqT", (D, S), bf16, kind="ExternalInput").ap()
    xkT = nc.dram_tensor("xkT", (D, S), bf16, kind="ExternalInput").ap()
    xvT = nc.dram_tensor("xvT", (D, S), bf16, kind="ExternalInput").ap()
    wq = nc.dram_tensor("wq", (D, D), bf16, kind="ExternalInput").ap()
    wk = nc.dram_tensor("wk", (D, D), bf16, kind="ExternalInput").ap()
    wv = nc.dram_tensor("wv", (D, D), bf16, kind="ExternalInput").ap()
    # valid01[k, q] = 1.0 where key k is valid for sorted query q, else 0.0
    maskT = nc.dram_tensor("maskT", (S, S), bf16, kind="ExternalInput").ap()
    out = nc.dram_tensor("out", (S, D), bf16, kind="ExternalOutput").ap()

    with ExitStack() as ctx:
        tc = ctx.enter_context(tile.TileContext(nc))
        const = ctx.enter_context(tc.tile_pool(name="const", bufs=1))
        persist = ctx.enter_context(tc.tile_pool(name="persist", bufs=1))
        wpool = ctx.enter_context(tc.tile_pool(name="wpool", bufs=1))
        ppool = ctx.enter_context(tc.tile_pool(name="ppool", bufs=1, space="PSUM"))
        epool = ctx.enter_context(tc.tile_pool(name="epool", bufs=6))
        mpool = ctx.enter_context(tc.tile_pool(name="mpool", bufs=3))

        NB = 512  # max psum-bank columns (fp32) per matmul

        def mm(out_ap, lhsT, rhs, base, start, stop):
            # split a wide matmul into <=512-col pieces so each PE write
            # stays inside one PSUM bank. base = column offset of out_ap[0]
            # within its tile (bank alignment reference).
            w = rhs.shape[-1]
            off = 0
            while off < w:
                step = min(NB - ((base + off) % NB), w - off)
                nc.tensor.matmul(
                    out_ap[:, off : off + step],
                    lhsT,
                    rhs[:, off : off + step],
                    start=start,
                    stop=stop,
                )
                off += step

        ident = const.tile([P, P], fp32)
        make_identity(nc, ident[:])

        rep_cm = tc.For_i(0, reps, 1) if reps > 1 else None
        if rep_cm is not None:
            ctx.enter_context(rep_cm)

        qt_sb = [persist.tile([P, S], bf16, tag=f"qt{i}", name=f"qt{i}") for i in range(DC)]
        kt_sb = [persist.tile([P, S], bf16, tag=f"kt{i}", name=f"kt{i}") for i in range(DC)]
        va_sb = [persist.tile([P, H * (DH + 1)], bf16, tag=f"va{i}", name=f"va{i}") for i in range(KC)]
        mk_sb = [persist.tile([P, S], bf16, tag=f"mk{i}", name=f"mk{i}") for i in range(KC)]

        # ---- input loads. x and W fully resident (bf16, 2MB each) ----
        def load_x(x_dram, pfx):
            return [wpool.tile([P, S], bf16, tag=f"x{pfx}{i}", name=f"x{pfx}{i}") for i in range(DC)]

        def load_w(w_dram, pfx):
            return [wpool.tile([P, D], bf16, tag=f"w{pfx}{i}", name=f"w{pfx}{i}") for i in range(DC)]

        xq_sb, wq_sb = load_x(xqT, "q"), load_w(wq, "q")
        xk_sb, wk_sb = load_x(xkT, "k"), load_w(wk, "k")
        xv_sb, wv_sb = load_x(xvT, "v"), load_w(wv, "v")
        # interleave x/w chunk loads so the first proj matmul can start
        # after two DMAs; Q/K first (unblocks pair-0 scores+exp), then V.
        for dc in range(DC):
            nc.sync.dma_start(xq_sb[dc][:], xqT[dc * P : (dc + 1) * P, :])
            nc.sync.dma_start(wq_sb[dc][:], wq[dc * P : (dc + 1) * P, :])
        for dc in range(DC):
            nc.sync.dma_start(xk_sb[dc][:], xkT[dc * P : (dc + 1) * P, :])
            nc.sync.dma_start(wk_sb[dc][:], wk[dc * P : (dc + 1) * P, :])
        for dc in range(DC):
            nc.sync.dma_start(xv_sb[dc][:], xvT[dc * P : (dc + 1) * P, :])
            nc.sync.dma_start(wv_sb[dc][:], wv[dc * P : (dc + 1) * P, :])
        for kc in range(KC):
            nc.sync.dma_start(mk_sb[kc][:], maskT[kc * P : (kc + 1) * P, :])
            va3 = va_sb[kc].rearrange("p (h d) -> p h d", d=DH + 1)
            nc.vector.memset(va3[:, :, DH], 1.0)

        # ---- projections (sequential phase; 4-tag 4-deep acc rotation) ----
        PTAGS = ["pA", "pB", "aA", "aB"]
        pt = [0]

        def acc_tile():
            t = ppool.tile([P, S], fp32, tag=PTAGS[pt[0] % 4], name="acc")
            pt[0] += 1
            return t

        def project_t(w_sb, xf, dst_sb, evac_engine):
            # out[d, q] = W^T @ xT ; per out-chunk: acc[128, 1024] over dc
            for oc in range(DC):
                acc = acc_tile()
                for dc in range(DC):
                    if "nomm" in ABLATE:
                        break
                    mm(acc[:], w_sb[dc][:, oc * P : (oc + 1) * P], xf[dc][:],
                       0, dc == 0, dc == DC - 1)
                if "nomm" not in ABLATE:
                    if evac_engine == "act":
                        nc.scalar.copy(dst_sb[oc][:], acc[:])
                    else:
                        nc.vector.tensor_copy(dst_sb[oc][:], acc[:])

        if "noproj" not in ABLATE:
            project_t(wq_sb, xq_sb, qt_sb, "act")
            project_t(wk_sb, xk_sb, kt_sb, "dve")
            # V: out[k, d] tiles; lhsT = xvT chunk [hid, k], rhs = Wv natural
            for kc in range(KC):
                acc = acc_tile()
                for dc in range(DC):
                    if "nomm" in ABLATE:
                        break
                    mm(acc[:], xv_sb[dc][:, kc * P : (kc + 1) * P], wv_sb[dc][:],
                       0, dc == 0, dc == DC - 1)
                if "nomm" not in ABLATE:
                    dst = va_sb[kc].rearrange("p (h d) -> p h d", d=DH + 1)[:, :, 0:DH]
                    nc.scalar.copy(dst, acc[:].rearrange("p (h d) -> p h d", d=DH))

        # ---- attention, head pairs ----
        kcs = [kc for kc in range(KC) if col_start[kc] < S]
        outv = out.rearrange("(s p) d -> p s d", p=P)  # [128, KC, D]

        def finalize(h, att, tag):
            # att rows 0:64 = O^T unnormalized, row 64 = Z
            asb = mpool.tile([DH + 1, S], fp32, tag="asb")
            nc.vector.tensor_copy(asb[:], att[:])
            # transpose staging reuses the accumulator's PSUM slot (released
            # by the asb copy); per-chunk stride 128 keeps each transpose
            # output inside one PSUM bank.
            trs = ppool.tile([P, KC * TRS], fp32, tag=tag, name="trs")
            tr3 = trs.rearrange("p (s d) -> p s d", d=TRS)
            for s in range(KC):
                nc.tensor.transpose(
                    tr3[:, s, 0 : DH + 1],
                    asb[:, s * P : (s + 1) * P],
                    ident[: DH + 1, : DH + 1],
                )
            rz = mpool.tile([P, KC], fp32, tag="rz")
            nc.vector.reciprocal(rz[:], tr3[:, :, DH])
            ot = mpool.tile([P, KC * DH], bf16, tag="ot")
            ot3 = ot.rearrange("p (s d) -> p s d", d=DH)
            for s in range(KC):
                nc.vector.tensor_scalar_mul(ot3[:, s, :], tr3[:, s, 0:DH], rz[:, s : s + 1])
            if "nodma" not in ABLATE:
                nc.gpsimd.dma_start(outv[:, :, h * DH : (h + 1) * DH], ot3[:])

        for oc in (range(DC) if "noattn" not in ABLATE else []):
            he, ho = 2 * oc, 2 * oc + 1
            att_e = ppool.tile([DH + 1, S], fp32, tag="aA", name="att_e")
            att_o = ppool.tile([DH + 1, S], fp32, tag="aB", name="att_o")
            for i, kc in enumerate(kcs):
                c0 = col_start[kc]
                cv = pred_end[kc]
                sc_e = ppool.tile([P, S], fp32, tag="pA", name="sc_e")
                sc_o = ppool.tile([P, S], fp32, tag="pB", name="sc_o")
                # paired scores: K=64 at PE rows 0:64 and 64:128 -> the two
                # matmuls row-tile and execute concurrently on the array.
                mm(sc_e[:, c0:], kt_sb[oc][0:DH, kc * P : (kc + 1) * P],
                   qt_sb[oc][0:DH, c0:], c0, True, True)
                mm(sc_o[:, c0:], kt_sb[oc][DH:P, kc * P : (kc + 1) * P],
                   qt_sb[oc][DH:P, c0:], c0, True, True)
                e_e = epool.tile([P, S], bf16, tag="e")
                e_o = epool.tile([P, S], bf16, tag="e")
                # exp first (releases the scores PSUM slot), then zero the
                # masked positions by multiplying with the 0/1 bf16 mask --
                # off the scores-recycle chain and cheap in bf16/SBUF.
                nc.scalar.activation(e_e[:, c0:], sc_e[:, c0:], AF.Exp, scale=0.125)
                nc.scalar.activation(e_o[:, c0:], sc_o[:, c0:], AF.Exp, scale=0.125)
                if cv > c0 and "nopred" not in ABLATE:
                    nc.vector.tensor_mul(
                        e_e[:, c0:cv], e_e[:, c0:cv], mk_sb[kc][:, c0:cv]
                    )
                    nc.vector.tensor_mul(
                        e_o[:, c0:cv], e_o[:, c0:cv], mk_sb[kc][:, c0:cv]
                    )
                mm(att_e[:, c0:], va_sb[kc][:, he * (DH + 1) : (he + 1) * (DH + 1)],
                   e_e[:, c0:], c0, i == 0, i == len(kcs) - 1)
                mm(att_o[:, c0:], va_sb[kc][:, ho * (DH + 1) : (ho + 1) * (DH + 1)],
                   e_o[:, c0:], c0, i == 0, i == len(kcs) - 1)
            if "notr" not in ABLATE:
                finalize(he, att_e, "aA")
                finalize(ho, att_o, "aB")


    nc.compile()
    return nc


_NC_CACHE = {}
_RUNNER_CACHE = {}
_PREP_JIT = []
_LAST_IN_MAPS = None


def _get_nc(col_start, pred_end):
    key = (tuple(col_start), tuple(pred_end))
    if key not in _NC_CACHE:
        _NC_CACHE[key] = _build_nc(list(col_start), list(pred_end))
    return _NC_CACHE[key]


def _get_runner(nc):
    """Build the sharded PJRT callable ONCE per nc and reuse it across
    kernel() calls -- run_bass_kernel_spmd re-traces and re-jits on every
    invocation, which costs seconds of host time per call."""
    if nc in _RUNNER_CACHE:
        return _RUNNER_CACHE[nc]
    import jax
    import concourse.mybir as mybir
    from jax.sharding import Mesh, PartitionSpec
    from jax.experimental.shard_map import shard_map
    from concourse import bass2jax

    bass2jax.install_neuronx_cc_hook()
    partition_name = nc.partition_id_tensor.name if nc.partition_id_tensor else None
    in_names, out_names, out_avals = [], [], []
    for alloc in nc.m.functions[0].allocations:
        if not isinstance(alloc, mybir.MemoryLocationSet):
            continue
        if not alloc.memorylocations:
            continue
        name = alloc.memorylocations[0].name
        if alloc.kind == "ExternalInput":
            if name != partition_name:
                in_names.append(name)
        elif alloc.kind == "ExternalOutput":
            out_names.append(name)
            shape = tuple(alloc.tensor_shape)
            dtype = mybir.dt.np(alloc.dtype)
            out_avals.append(jax.core.ShapedArray(shape, dtype))
    n_params = len(in_names)
    all_in = in_names + out_names + ([partition_name] if partition_name else [])

    def _body(*args):
        operands = list(args)
        if partition_name is not None:
            operands.append(bass2jax.partition_id_tensor())
        outs = bass2jax._bass_exec_p.bind(
            *operands,
            out_avals=tuple(out_avals),
            in_names=tuple(all_in),
            out_names=tuple(out_names),
            lowering_input_output_aliases=(),
            sim_require_finite=True,
            sim_require_nnan=True,
            nc=nc,
        )
        return tuple(outs)

    devices = jax.devices()[:N_CORES]
    mesh = Mesh(np.asarray(devices), ("core",))
    n_outs = len(out_names)
    sharded = jax.jit(
        shard_map(
            _body,
            mesh=mesh,
            in_specs=(PartitionSpec("core"),) * (n_params + n_outs),
            out_specs=(PartitionSpec("core"),) * n_outs,
            check_rep=False,
        ),
        keep_unused=True,
    )
    # every element of every output is written by the kernel, so the
    # pre-zeroed output operands can be device-resident and reused.
    zeros = [
        jax.device_put(
            np.zeros((N_CORES * a.shape[0], *a.shape[1:]), a.dtype)
        )
        for a in out_avals
    ]
    for z in zeros:
        z.block_until_ready()

    def run(in_maps):
        concat_in = [
            np.concatenate([np.asarray(m[name]) for m in in_maps], axis=0)
            for name in in_names
        ]
        out_arrs = sharded(*concat_in, *zeros)
        return [
            {
                name: np.asarray(out_arrs[i]).reshape(
                    N_CORES, *out_avals[i].shape
                )[c]
                for i, name in enumerate(out_names)
            }
            for c in range(N_CORES)
        ]

    _RUNNER_CACHE[nc] = run
    return run


def _get_prep_jit():
    """jax-CPU jitted input prep (transpose + bf16 cast + mask build) --
    multi-threaded XLA beats the serial numpy path by ~5x."""
    if not _PREP_JIT:
        import jax
        import jax.numpy as jnp

        cpu = jax.devices("cpu")[0]

        def f(q, k, v, orders, vs):
            qs = jnp.take_along_axis(q, orders[:, :, None], axis=1)
            xqT = jnp.swapaxes(qs, 1, 2).astype(jnp.bfloat16)
            xkT = jnp.swapaxes(k, 1, 2).astype(jnp.bfloat16)
            xvT = jnp.swapaxes(v, 1, 2).astype(jnp.bfloat16)
            kidx = jnp.arange(S, dtype=jnp.int32)
            maskT = (kidx[None, :, None] < vs[:, None, :]).astype(jnp.bfloat16)
            return xqT, xkT, xvT, maskT

        def fo(o_bf, inv):
            # bf16 device output -> fp32, unsorted back to query order
            return jnp.take_along_axis(
                o_bf.astype(jnp.float32), inv[:, :, None], axis=1
            )

        fj, foj = jax.jit(f), jax.jit(fo)

        def fin(*a):
            with jax.default_device(cpu):
                return fj(*a)

        def fout(*a):
            with jax.default_device(cpu):
                return foj(*a)

        _PREP_JIT.append(fin)
        _PREP_JIT.append(fout)
    return _PREP_JIT


def _prep(query, key, value, valid_len, Wq, Wk, Wv):
    import ml_dtypes

    bf = ml_dtypes.bfloat16
    orders = []
    vss = []
    col_start = [S] * KC
    pred_end = [0] * KC
    wqb, wkb, wvb = Wq.astype(bf), Wk.astype(bf), Wv.astype(bf)
    for b in range(B):
        vl = valid_len[b]
        vl2 = np.where(vl == 0, 1, vl).astype(np.int32)
        order = np.argsort(vl2, kind="stable")
        orders.append(order)
        vs = vl2[order]
        vss.append(vs)
        for kc in range(KC):
            need = vs > (kc * P)
            c0 = S if not need.any() else (int(np.argmax(need)) // 32) * 32
            col_start[kc] = min(col_start[kc], c0)
            full = vs >= ((kc + 1) * P)
            cv = S if not full.any() else int(np.argmax(full))
            pred_end[kc] = max(pred_end[kc], min(S, -(-cv // 32) * 32))
    fin, _ = _get_prep_jit()
    xqT, xkT, xvT, maskT = (
        np.asarray(a, dtype=bf)
        for a in fin(
            query, key, value,
            np.stack(orders).astype(np.int32),
            np.stack(vss).astype(np.int32),
        )
    )
    in_maps = [
        {
            "xqT": xqT[b],
            "xkT": xkT[b],
            "xvT": xvT[b],
            "wq": wqb,
            "wk": wkb,
            "wv": wvb,
            "maskT": maskT[b],
        }
        for b in range(B)
    ]
    return in_maps, orders, col_start, pred_end


def kernel(query, key, value, valid_len, Wq, Wk, Wv):
    query = np.asarray(query, dtype=np.float32)
    key = np.asarray(key, dtype=np.float32)
    value = np.asarray(value, dtype=np.float32)
    valid_len = np.asarray(valid_len, dtype=np.int32)
    Wq = np.asarray(Wq, dtype=np.float32)
    Wk = np.asarray(Wk, dtype=np.float32)
    Wv = np.asarray(Wv, dtype=np.float32)

    in_maps, orders, col_start, pred_end = _prep(
        query, key, value, valid_len, Wq, Wk, Wv
    )
    nc = _get_nc(col_start, pred_end)
    global _LAST_IN_MAPS
    _LAST_IN_MAPS = in_maps
    results = _get_runner(nc)(in_maps)

    _, fout = _get_prep_jit()
    o_stack = np.stack([results[b]["out"] for b in range(B)])
    invs = np.empty((B, S), dtype=np.int32)
    for b in range(B):
        invs[b][orders[b]] = np.arange(S, dtype=np.int32)
    outs = np.array(fout(o_stack, invs), dtype=np.float32)
    for b in range(B):
        zrows = np.where(valid_len[b] == 0)[0]
        if len(zrows):
            outs[b][zrows] = value[b].mean(axis=0) @ Wv
    return outs


# revision 33
# speedup vs baseline: 1.0489x; 1.0489x over previous
"""MHA (projections + masked softmax attention) on 8 NeuronCores.

Data-parallel over batch (B=8 -> 1 batch element per core, no collectives).
bf16 matmul operands (fp32 PSUM accumulation + fp32 softmax normalization).

Per core, transposed layout:
  QT = Wq^T @ x_q^T   [D, Sq]   (lhsT = Wq natural, rhs = x_q^T from host)
  KT = Wk^T @ x_k^T   [D, Sk]
  V  = x_v  @ Wv      [Sk, D]   (lhsT = x_v^T chunk, rhs = Wv natural)

Attention processed in head PAIRS (2h, 2h+1) sharing one 128-partition
qt/kt chunk: the two heads' scores matmuls have K=64 contraction at PE
row offsets 0 and 64 (tile_position row-tiling) and execute concurrently
on the PE array. Scores in "transposed" layout S^T[k, q]:
  S^T = KT_h_chunk.T @ QT_h                (k on partitions, q free)
  masking: e = exp(0.125*s) first, then e *= valid01 (bf16 0/1 mask) on
  the ragged [c0:cv) column range only -- off the PSUM-recycle chain.
  O^T[d,q] & Z[q] in ONE accumulating matmul: lhsT = [V_h | ones] (65 cols)
  final: O = transpose(O^T) * (1/Z) per 128-query block; ONE strided
  bf16 DMA per head (issued from the Pool engine DGE to keep SP free).

Host: transposes, sort queries by valid_len (column-suffix skipping of
fully-masked key chunks + narrow predication ranges), uint8 mask, exact
fixup of valid_len==0 rows (reference -> uniform softmax -> mean(value)@Wv).
"""

import os
import sys

if "/opt/trn_rl_repo" not in sys.path:
    sys.path.insert(0, "/opt/trn_rl_repo")

import numpy as np

ABLATE = set(os.environ.get("ABLATE", "").split(","))

B, S, D, H = 8, 1024, 1024, 16
DH = D // H  # 64
P = 128
KC = S // P  # 8 key chunks
DC = D // P  # 8 hidden chunks
N_CORES = 8
NEG = -480.0  # exp(0.125 * -480) = exp(-60) ~= 8.8e-27
TRS = 128  # per-chunk column stride in the transpose staging tile


def _build_nc(col_start, pred_end, reps=1):
    """col_start[kc]: first sorted-q column (mult of 128, 0..1024) needing
    key-chunk kc (1024 = chunk skipped). pred_end[kc]: end (exclusive, mult
    of 32) of the mask-predication range. Unions over cores. col_start[0]
    must be 0."""
    from contextlib import ExitStack

    import concourse.mybir as mybir
    import concourse.tile as tile
    from concourse import bacc
    from concourse.masks import make_identity

    fp32 = mybir.dt.float32
    bf16 = mybir.dt.bfloat16
    u8 = mybir.dt.uint8
    AF = mybir.ActivationFunctionType

    nc = bacc.Bacc(
        "TRN2",
        target_bir_lowering=False,
        debug=False,
        enable_asserts=False,
        num_devices=N_CORES,
    )

    xqT = nc.dram_tensor("xqT", (D, S), bf16, kind="ExternalInput").ap()
    xkT = nc.dram_tensor("xkT", (D, S), bf16, kind="ExternalInput").ap()
    xvT = nc.dram_tensor("xvT", (D, S), bf16, kind="ExternalInput").ap()
    wq = nc.dram_tensor("wq", (D, D), bf16, kind="ExternalInput").ap()
    wk = nc.dram_tensor("wk", (D, D), bf16, kind="ExternalInput").ap()
    wv = nc.dram_tensor("wv", (D, D), bf16, kind="ExternalInput").ap()
    # valid01[k, q] = 1.0 where key k is valid for sorted query q, else 0.0
    maskT = nc.dram_tensor("maskT", (S, S), bf16, kind="ExternalInput").ap()
    out = nc.dram_tensor("out", (S, D), bf16, kind="ExternalOutput").ap()

    with ExitStack() as ctx:
        tc = ctx.enter_context(tile.TileContext(nc))
        const = ctx.enter_context(tc.tile_pool(name="const", bufs=1))
        persist = ctx.enter_context(tc.tile_pool(name="persist", bufs=1))
        wpool = ctx.enter_context(tc.tile_pool(name="wpool", bufs=1))
        ppool = ctx.enter_context(tc.tile_pool(name="ppool", bufs=1, space="PSUM"))
        epool = ctx.enter_context(tc.tile_pool(name="epool", bufs=6))
        mpool = ctx.enter_context(tc.tile_pool(name="mpool", bufs=3))

        NB = 512  # max psum-bank columns (fp32) per matmul

        def mm(out_ap, lhsT, rhs, base, start, stop):
            # split a wide matmul into <=512-col pieces so each PE write
            # stays inside one PSUM bank. base = column offset of out_ap[0]
            # within its tile (bank alignment reference).
            w = rhs.shape[-1]
            off = 0
            while off < w:
                step = min(NB - ((base + off) % NB), w - off)
                nc.tensor.matmul(
                    out_ap[:, off : off + step],
                    lhsT,
                    rhs[:, off : off + step],
                    start=start,
                    stop=stop,
                )
                off += step

        ident = const.tile([P, P], fp32)
        make_identity(nc, ident[:])

        rep_cm = tc.For_i(0, reps, 1) if reps > 1 else None
        if rep_cm is not None:
            ctx.enter_context(rep_cm)

        qt_sb = [persist.tile([P, S], bf16, tag=f"qt{i}", name=f"qt{i}") for i in range(DC)]
        kt_sb = [persist.tile([P, S], bf16, tag=f"kt{i}", name=f"kt{i}") for i in range(DC)]
        va_sb = [persist.tile([P, H * (DH + 1)], bf16, tag=f"va{i}", name=f"va{i}") for i in range(KC)]
        mk_sb = [persist.tile([P, S], bf16, tag=f"mk{i}", name=f"mk{i}") for i in range(KC)]

        # ---- input loads. x and W fully resident (bf16, 2MB each) ----
        def load_x(x_dram, pfx):
            return [wpool.tile([P, S], bf16, tag=f"x{pfx}{i}", name=f"x{pfx}{i}") for i in range(DC)]

        def load_w(w_dram, pfx):
            return [wpool.tile([P, D], bf16, tag=f"w{pfx}{i}", name=f"w{pfx}{i}") for i in range(DC)]

        xq_sb, wq_sb = load_x(xqT, "q"), load_w(wq, "q")
        xk_sb, wk_sb = load_x(xkT, "k"), load_w(wk, "k")
        xv_sb, wv_sb = load_x(xvT, "v"), load_w(wv, "v")
        # interleave x/w chunk loads so the first proj matmul can start
        # after two DMAs; Q/K first (unblocks pair-0 scores+exp), then V.
        for dc in range(DC):
            nc.sync.dma_start(xq_sb[dc][:], xqT[dc * P : (dc + 1) * P, :])
            nc.sync.dma_start(wq_sb[dc][:], wq[dc * P : (dc + 1) * P, :])
        for dc in range(DC):
            nc.sync.dma_start(xk_sb[dc][:], xkT[dc * P : (dc + 1) * P, :])
            nc.sync.dma_start(wk_sb[dc][:], wk[dc * P : (dc + 1) * P, :])
        for dc in range(DC):
            nc.sync.dma_start(xv_sb[dc][:], xvT[dc * P : (dc + 1) * P, :])
            nc.sync.dma_start(wv_sb[dc][:], wv[dc * P : (dc + 1) * P, :])
        for kc in range(KC):
            nc.sync.dma_start(mk_sb[kc][:], maskT[kc * P : (kc + 1) * P, :])
            va3 = va_sb[kc].rearrange("p (h d) -> p h d", d=DH + 1)
            nc.vector.memset(va3[:, :, DH], 1.0)

        # ---- projections ----
        # accs use ONLY the aA/aB rings (2-deep pipeline: evac ~1.1us <
        # 3.4us of accumulating matmuls) so the pA/pB scores rings stay
        # free and early head pairs can run scores+exp DURING projections,
        # buffering results in the (deep) e-tile pool.
        pt = [0]

        def acc_tile():
            t = ppool.tile([P, S], fp32, tag="aA" if pt[0] % 2 == 0 else "aB",
                           name="acc")
            pt[0] += 1
            return t

        def project_t(w_sb, xf, dst_sb, evac_engine):
            # out[d, q] = W^T @ xT ; per out-chunk: acc[128, 1024] over dc
            for oc in range(DC):
                acc = acc_tile()
                for dc in range(DC):
                    if "nomm" in ABLATE:
                        break
                    mm(acc[:], w_sb[dc][:, oc * P : (oc + 1) * P], xf[dc][:],
                       0, dc == 0, dc == DC - 1)
                if "nomm" not in ABLATE:
                    if evac_engine == "act":
                        nc.scalar.copy(dst_sb[oc][:], acc[:])
                    else:
                        nc.vector.tensor_copy(dst_sb[oc][:], acc[:])

        if "noproj" not in ABLATE:
            project_t(wq_sb, xq_sb, qt_sb, "act")
            project_t(wk_sb, xk_sb, kt_sb, "dve")
            # V: out[k, d] tiles; lhsT = xvT chunk [hid, k], rhs = Wv natural
            for kc in range(KC):
                acc = acc_tile()
                for dc in range(DC):
                    if "nomm" in ABLATE:
                        break
                    mm(acc[:], xv_sb[dc][:, kc * P : (kc + 1) * P], wv_sb[dc][:],
                       0, dc == 0, dc == DC - 1)
                if "nomm" not in ABLATE:
                    dst = va_sb[kc].rearrange("p (h d) -> p h d", d=DH + 1)[:, :, 0:DH]
                    nc.scalar.copy(dst, acc[:].rearrange("p (h d) -> p h d", d=DH))

        # ---- attention, head pairs ----
        kcs = [kc for kc in range(KC) if col_start[kc] < S]
        outv = out.rearrange("(s p) d -> p s d", p=P)  # [128, KC, D]

        # deep e-tile buffering: rotate over the epool slots plus the
        # xq/xk staging slots (same shape/dtype), which go dead as soon as
        # the Q/K projections finish reading them. Lets ACT run exp well
        # ahead of the (projection-gated) AV matmuls.
        et = [0]

        def e_tile():
            r = et[0] % 22
            et[0] += 1
            if r < 6:
                return epool.tile([P, S], bf16, tag="e", name="e")
            r -= 6
            tag = f"xq{r}" if r < 8 else f"xk{r - 8}"
            return wpool.tile([P, S], bf16, tag=tag, name=f"e_{tag}")

        def finalize(h, att, tag):
            # att rows 0:64 = O^T unnormalized, row 64 = Z
            asb = mpool.tile([DH + 1, S], fp32, tag="asb")
            nc.vector.tensor_copy(asb[:], att[:])
            # transpose staging reuses the accumulator's PSUM slot (released
            # by the asb copy); per-chunk stride 128 keeps each transpose
            # output inside one PSUM bank.
            trs = ppool.tile([P, KC * TRS], fp32, tag=tag, name="trs")
            tr3 = trs.rearrange("p (s d) -> p s d", d=TRS)
            for s in range(KC):
                nc.tensor.transpose(
                    tr3[:, s, 0 : DH + 1],
                    asb[:, s * P : (s + 1) * P],
                    ident[: DH + 1, : DH + 1],
                )
            rz = mpool.tile([P, KC], fp32, tag="rz")
            nc.vector.reciprocal(rz[:], tr3[:, :, DH])
            ot = mpool.tile([P, KC * DH], bf16, tag="ot")
            ot3 = ot.rearrange("p (s d) -> p s d", d=DH)
            for s in range(KC):
                nc.vector.tensor_scalar_mul(ot3[:, s, :], tr3[:, s, 0:DH], rz[:, s : s + 1])
            if "nodma" not in ABLATE:
                # SP's HWDGE queue is idle once the input loads finish
                nc.sync.dma_start(outv[:, :, h * DH : (h + 1) * DH], ot3[:])

        for oc in (range(DC) if "noattn" not in ABLATE else []):
            he, ho = 2 * oc, 2 * oc + 1
            att_e = ppool.tile([DH + 1, S], fp32, tag="aA", name="att_e")
            att_o = ppool.tile([DH + 1, S], fp32, tag="aB", name="att_o")
            for i, kc in enumerate(kcs):
                c0 = col_start[kc]
                cv = pred_end[kc]
                sc_e = ppool.tile([P, S], fp32, tag="pA", name="sc_e")
                sc_o = ppool.tile([P, S], fp32, tag="pB", name="sc_o")
                # paired scores: K=64 at PE rows 0:64 and 64:128 -> the two
                # matmuls row-tile and execute concurrently on the array.
                mm(sc_e[:, c0:], kt_sb[oc][0:DH, kc * P : (kc + 1) * P],
                   qt_sb[oc][0:DH, c0:], c0, True, True)
                mm(sc_o[:, c0:], kt_sb[oc][DH:P, kc * P : (kc + 1) * P],
                   qt_sb[oc][DH:P, c0:], c0, True, True)
                e_e = e_tile()
                e_o = e_tile()
                # exp first (releases the scores PSUM slot), then zero the
                # masked positions by multiplying with the 0/1 bf16 mask --
                # off the scores-recycle chain and cheap in bf16/SBUF.
                nc.scalar.activation(e_e[:, c0:], sc_e[:, c0:], AF.Exp, scale=0.125)
                nc.scalar.activation(e_o[:, c0:], sc_o[:, c0:], AF.Exp, scale=0.125)
                if cv > c0 and "nopred" not in ABLATE:
                    nc.vector.tensor_mul(
                        e_e[:, c0:cv], e_e[:, c0:cv], mk_sb[kc][:, c0:cv]
                    )
                    nc.vector.tensor_mul(
                        e_o[:, c0:cv], e_o[:, c0:cv], mk_sb[kc][:, c0:cv]
                    )
                mm(att_e[:, c0:], va_sb[kc][:, he * (DH + 1) : (he + 1) * (DH + 1)],
                   e_e[:, c0:], c0, i == 0, i == len(kcs) - 1)
                mm(att_o[:, c0:], va_sb[kc][:, ho * (DH + 1) : (ho + 1) * (DH + 1)],
                   e_o[:, c0:], c0, i == 0, i == len(kcs) - 1)
            if "notr" not in ABLATE:
                finalize(he, att_e, "aA")
                finalize(ho, att_o, "aB")


    nc.compile()
    return nc


_NC_CACHE = {}
_RUNNER_CACHE = {}
_PREP_JIT = []
_LAST_IN_MAPS = None


def _get_nc(col_start, pred_end):
    key = (tuple(col_start), tuple(pred_end))
    if key not in _NC_CACHE:
        _NC_CACHE[key] = _build_nc(list(col_start), list(pred_end))
    return _NC_CACHE[key]


def _get_runner(nc):
    """Build the sharded PJRT callable ONCE per nc and reuse it across
    kernel() calls -- run_bass_kernel_spmd re-traces and re-jits on every
    invocation, which costs seconds of host time per call."""
    if nc in _RUNNER_CACHE:
        return _RUNNER_CACHE[nc]
    import jax
    import concourse.mybir as mybir
    from jax.sharding import Mesh, PartitionSpec
    from jax.experimental.shard_map import shard_map
    from concourse import bass2jax

    bass2jax.install_neuronx_cc_hook()
    partition_name = nc.partition_id_tensor.name if nc.partition_id_tensor else None
    in_names, out_names, out_avals = [], [], []
    for alloc in nc.m.functions[0].allocations:
        if not isinstance(alloc, mybir.MemoryLocationSet):
            continue
        if not alloc.memorylocations:
            continue
        name = alloc.memorylocations[0].name
        if alloc.kind == "ExternalInput":
            if name != partition_name:
                in_names.append(name)
        elif alloc.kind == "ExternalOutput":
            out_names.append(name)
            shape = tuple(alloc.tensor_shape)
            dtype = mybir.dt.np(alloc.dtype)
            out_avals.append(jax.core.ShapedArray(shape, dtype))
    n_params = len(in_names)
    all_in = in_names + out_names + ([partition_name] if partition_name else [])

    def _body(*args):
        operands = list(args)
        if partition_name is not None:
            operands.append(bass2jax.partition_id_tensor())
        outs = bass2jax._bass_exec_p.bind(
            *operands,
            out_avals=tuple(out_avals),
            in_names=tuple(all_in),
            out_names=tuple(out_names),
            lowering_input_output_aliases=(),
            sim_require_finite=True,
            sim_require_nnan=True,
            nc=nc,
        )
        return tuple(outs)

    devices = jax.devices()[:N_CORES]
    mesh = Mesh(np.asarray(devices), ("core",))
    n_outs = len(out_names)
    sharded = jax.jit(
        shard_map(
            _body,
            mesh=mesh,
            in_specs=(PartitionSpec("core"),) * (n_params + n_outs),
            out_specs=(PartitionSpec("core"),) * n_outs,
            check_rep=False,
        ),
        keep_unused=True,
    )
    # every element of every output is written by the kernel, so the
    # pre-zeroed output operands can be device-resident and reused.
    zeros = [
        jax.device_put(
            np.zeros((N_CORES * a.shape[0], *a.shape[1:]), a.dtype)
        )
        for a in out_avals
    ]
    for z in zeros:
        z.block_until_ready()

    def run(in_maps):
        concat_in = [
            np.concatenate([np.asarray(m[name]) for m in in_maps], axis=0)
            for name in in_names
        ]
        out_arrs = sharded(*concat_in, *zeros)
        return [
            {
                name: np.asarray(out_arrs[i]).reshape(
                    N_CORES, *out_avals[i].shape
                )[c]
                for i, name in enumerate(out_names)
            }
            for c in range(N_CORES)
        ]

    _RUNNER_CACHE[nc] = run
    return run


def _get_prep_jit():
    """jax-CPU jitted input prep (transpose + bf16 cast + mask build) --
    multi-threaded XLA beats the serial numpy path by ~5x."""
    if not _PREP_JIT:
        import jax
        import jax.numpy as jnp

        cpu = jax.devices("cpu")[0]

        def f(q, k, v, orders, vs):
            qs = jnp.take_along_axis(q, orders[:, :, None], axis=1)
            xqT = jnp.swapaxes(qs, 1, 2).astype(jnp.bfloat16)
            xkT = jnp.swapaxes(k, 1, 2).astype(jnp.bfloat16)
            xvT = jnp.swapaxes(v, 1, 2).astype(jnp.bfloat16)
            kidx = jnp.arange(S, dtype=jnp.int32)
            maskT = (kidx[None, :, None] < vs[:, None, :]).astype(jnp.bfloat16)
            return xqT, xkT, xvT, maskT

        def fo(o_bf, inv):
            # bf16 device output -> fp32, unsorted back to query order
            return jnp.take_along_axis(
                o_bf.astype(jnp.float32), inv[:, :, None], axis=1
            )

        fj, foj = jax.jit(f), jax.jit(fo)

        def fin(*a):
            with jax.default_device(cpu):
                return fj(*a)

        def fout(*a):
            with jax.default_device(cpu):
                return foj(*a)

        _PREP_JIT.append(fin)
        _PREP_JIT.append(fout)
    return _PREP_JIT


def _prep(query, key, value, valid_len, Wq, Wk, Wv):
    import ml_dtypes

    bf = ml_dtypes.bfloat16
    orders = []
    vss = []
    col_start = [S] * KC
    pred_end = [0] * KC
    wqb, wkb, wvb = Wq.astype(bf), Wk.astype(bf), Wv.astype(bf)
    for b in range(B):
        vl = valid_len[b]
        vl2 = np.where(vl == 0, 1, vl).astype(np.int32)
        order = np.argsort(vl2, kind="stable")
        orders.append(order)
        vs = vl2[order]
        vss.append(vs)
        for kc in range(KC):
            need = vs > (kc * P)
            c0 = S if not need.any() else (int(np.argmax(need)) // 32) * 32
            col_start[kc] = min(col_start[kc], c0)
            full = vs >= ((kc + 1) * P)
            cv = S if not full.any() else int(np.argmax(full))
            pred_end[kc] = max(pred_end[kc], min(S, -(-cv // 32) * 32))
    fin, _ = _get_prep_jit()
    xqT, xkT, xvT, maskT = (
        np.asarray(a, dtype=bf)
        for a in fin(
            query, key, value,
            np.stack(orders).astype(np.int32),
            np.stack(vss).astype(np.int32),
        )
    )
    in_maps = [
        {
            "xqT": xqT[b],
            "xkT": xkT[b],
            "xvT": xvT[b],
            "wq": wqb,
            "wk": wkb,
            "wv": wvb,
            "maskT": maskT[b],
        }
        for b in range(B)
    ]
    return in_maps, orders, col_start, pred_end


def kernel(query, key, value, valid_len, Wq, Wk, Wv):
    query = np.asarray(query, dtype=np.float32)
    key = np.asarray(key, dtype=np.float32)
    value = np.asarray(value, dtype=np.float32)
    valid_len = np.asarray(valid_len, dtype=np.int32)
    Wq = np.asarray(Wq, dtype=np.float32)
    Wk = np.asarray(Wk, dtype=np.float32)
    Wv = np.asarray(Wv, dtype=np.float32)

    in_maps, orders, col_start, pred_end = _prep(
        query, key, value, valid_len, Wq, Wk, Wv
    )
    nc = _get_nc(col_start, pred_end)
    global _LAST_IN_MAPS
    _LAST_IN_MAPS = in_maps
    results = _get_runner(nc)(in_maps)

    _, fout = _get_prep_jit()
    o_stack = np.stack([results[b]["out"] for b in range(B)])
    invs = np.empty((B, S), dtype=np.int32)
    for b in range(B):
        invs[b][orders[b]] = np.arange(S, dtype=np.int32)
    outs = np.array(fout(o_stack, invs), dtype=np.float32)
    for b in range(B):
        zrows = np.where(valid_len[b] == 0)[0]
        if len(zrows):
            outs[b][zrows] = value[b].mean(axis=0) @ Wv
    return outs
